# revision 6
# baseline (speedup 1.0000x reference)
"""Trainium2 Bass kernel for the shift-reduce recursive NN (nn_Recursive_44040594653244).

Reference semantics: a per-column shift-reduce stack machine over tokens
[511, 256].  setup_inputs() always emits the structure
    row 0:        shift token   (>= 3)
    rows 1,3,...: shift tokens  (>= 3)
    rows 2,4,...: CLOSE (reduce)
with no PAD/OPEN, which makes the computation a left-leaning chain per
batch column b:
    h = emb[tok0[b]]
    for i in 1..255:  h = tanh(h @ Wl + emb[tok_i[b]] @ Wr + bias)
    out[b] = h                                    # [256, 512] fp32

Device strategy (pure batch data-parallel, 8 cores x 32 columns):
  - indirect-DMA gather of the 8192 leaf-embedding rows per core
  - DVE cast fp32 -> fp16, PE transposes to a hidden-major layout
  - phase B: C^T tiles = Wr^T @ E^T for all steps (fp16 operands, fp32 psum)
  - phase C: 255 sequential steps with state kept transposed as
    hT [128 part = hid-within-k-tile, free = k-tile(4) x batch(32)]:
      psum = I @ cT_i  +  sum_k Wl[k,o-tile]^T @ hT[k]   (17 matmuls)
      hT   = tanh(psum)                                  (1 ACT op)
  - final PE transpose back to natural [32, 512] fp32

fp16 storage with fp32 psum accumulation gives ~2e-3 relative error vs
the fp32 reference (verified offline in numpy simulation).
"""

import numpy as np

H = 512            # hidden size
NT = 4             # 128-wide tiles in H
B_CORE = 32        # batch columns per core
N_CORES = 8
GROUPS = 64        # gather groups: 4 leaves x 32 batch = 128 rows per group
STEPS = 255        # sequential reduce steps
E32_RING = 8       # gathered fp32 ring ([128, 512] tiles)
E16_RING = 4       # fp16-cast ring
ET_RING = 4        # transposed fp16 ring

_cached = {}


def _build_nc():
    import concourse.bass as bass
    import concourse.mybir as mybir

    dt = mybir.dt
    nc = bass.Bass(target_bir_lowering=False)

    emb_d = nc.dram_tensor("emb", [32000, H], dt.float32, kind="ExternalInput")
    wl_d = nc.dram_tensor("wl16", [128, NT * H], dt.float16, kind="ExternalInput")
    wr_d = nc.dram_tensor("wr16", [128, NT * H], dt.float16, kind="ExternalInput")
    i16_d = nc.dram_tensor("i16", [128, 128], dt.float16, kind="ExternalInput")
    bt_d = nc.dram_tensor("bt", [128, NT], dt.float32, kind="ExternalInput")
    idx_d = nc.dram_tensor("idx", [128, GROUPS], dt.int32, kind="ExternalInput")
    y_d = nc.dram_tensor("y", [B_CORE, H], dt.float32, kind="ExternalOutput")

    from contextlib import ExitStack

    with ExitStack() as ctx:
        ent = ctx.enter_context
        wl_s = ent(nc.sbuf_tensor("wl16s", [128, NT * H], dt.float16))
        wr_s = ent(nc.sbuf_tensor("wr16s", [128, NT * H], dt.float16))
        i16_s = ent(nc.sbuf_tensor("i16s", [128, 128], dt.float16))
        bt_s = ent(nc.sbuf_tensor("bts", [128, NT], dt.float32))
        idx_s = ent(nc.sbuf_tensor("idxs", [128, GROUPS], dt.int32))
        e32_s = ent(nc.sbuf_tensor("e32", [128, E32_RING * H], dt.float32))
        e16_s = ent(nc.sbuf_tensor("e16", [128, E16_RING * H], dt.float16))
        et_s = ent(nc.sbuf_tensor("et16", [128, ET_RING * H], dt.float16))
        ct_s = ent(nc.sbuf_tensor("ct16", [128, GROUPS * H], dt.float16))
        ht_s = ent(nc.sbuf_tensor("ht16", [128, 2 * 128], dt.float16))
        y_s = ent(nc.sbuf_tensor("y32", [B_CORE, H], dt.float32))
        # each PSUM tensor is exactly one 2KB/partition bank
        # fp16 (transpose out dtype must match input); padded to a full bank
        ps_tr0 = ent(nc.psum_tensor("ps_tr0", [128, 1024], dt.float16))
        ps_tr1 = ent(nc.psum_tensor("ps_tr1", [128, 1024], dt.float16))
        ps_c0 = ent(nc.psum_tensor("ps_c0", [128, 512], dt.float32))
        ps_c1 = ent(nc.psum_tensor("ps_c1", [128, 512], dt.float32))
        ps_h0 = ent(nc.psum_tensor("ps_h0", [128, 512], dt.float32))
        ps_h1 = ent(nc.psum_tensor("ps_h1", [128, 512], dt.float32))
        ps_f = ent(nc.psum_tensor("ps_f", [128, 1024], dt.float16))
        s_pre = ent(nc.semaphore("s_pre"))     # input DMAs (sync), +16 each
        s_gth = ent(nc.semaphore("s_gth"))     # gathers (gpsimd), +16 each
        s_cast = ent(nc.semaphore("s_cast"))   # DVE fp16 cast, +1 per group
        s_trA = ent(nc.semaphore("s_trA"))     # PE transposes, +1 per group
        s_eT = ent(nc.semaphore("s_eT"))       # DVE eT copy, +1 per group
        s_bb = ent(nc.semaphore("s_bb"))       # PE phase-B MMs, +1 per group
        s_ct = ent(nc.semaphore("s_ct"))       # DVE CT copy, +1 per group
        s_mm = ent(nc.semaphore("s_mm"))       # PE step MMs, +1 per step
        s_tanh = ent(nc.semaphore("s_tanh"))   # ACT tanh, +1 per step
        s_fdve = ent(nc.semaphore("s_fdve"))
        s_done = ent(nc.semaphore("s_done"))
        block = ent(nc.Block())
        ps_tr = [ps_tr0, ps_tr1]
        ps_c = [ps_c0, ps_c1]
        ps_h = [ps_h0, ps_h1]

        def e32b(g):
            return (g % E32_RING) * H

        def e16b(g):
            return (g % E16_RING) * H

        def etb(g):
            return (g % ET_RING) * H

        # ------------- sync: input DMAs + final store -------------
        @block.sync
        def _(sync):
            sync.dma_start(idx_s[:, :], idx_d[:, :]).then_inc(s_pre, 16)
            sync.dma_start(i16_s[:, :], i16_d[:, :]).then_inc(s_pre, 16)
            sync.dma_start(wl_s[:, :], wl_d[:, :]).then_inc(s_pre, 16)
            sync.dma_start(wr_s[:, :], wr_d[:, :]).then_inc(s_pre, 16)
            sync.dma_start(bt_s[:, :], bt_d[:, :]).then_inc(s_pre, 16)
            sync.wait_ge(s_fdve, 1)
            sync.dma_start(y_d[:, :], y_s[:, :]).then_inc(s_done, 16)
            sync.wait_ge(s_done, 16)

        # ------------- gpsimd: indirect gathers -------------
        @block.gpsimd
        def _(gpsimd):
            gpsimd.wait_ge(s_pre, 16)  # idx loaded
            for g in range(GROUPS):
                if g >= E32_RING:
                    gpsimd.wait_ge(s_cast, g - E32_RING + 1)
                gpsimd.indirect_dma_start(
                    out=e32_s[:, e32b(g):e32b(g) + H],
                    out_offset=None,
                    in_=emb_d[:, :],
                    in_offset=bass.IndirectOffsetOnAxis(ap=idx_s[:, g:g + 1], axis=0),
                ).then_inc(s_gth, 16)

        # ------------- tensor engine -------------
        @block.tensor
        def _(tensor):
            tensor.wait_ge(s_pre, 80)  # all inputs resident

            def transposes(g):
                # E16[g] [128 flat, 512 hid] -> psumT [128 hid-in-k, 4k x 128 flat]
                tensor.wait_ge(s_cast, g + 1)
                if g >= 2:
                    tensor.wait_ge(s_eT, g - 1)  # DVE freed ps_tr[g%2]
                for j in range(NT):
                    mm = tensor.transpose(
                        out=ps_tr[g % 2][:, j * 128:(j + 1) * 128],
                        in_=e16_s[:, e16b(g) + j * 128:e16b(g) + (j + 1) * 128],
                        identity=i16_s[:, :],
                    )
                mm.then_inc(s_trA, 1)

            def phase_b(g):
                # CT_g[o-tile][128 o, 128 flat] = sum_k Wr[k,o]^T @ eT[k]
                tensor.wait_ge(s_eT, g + 1)
                if g >= 2:
                    tensor.wait_ge(s_ct, g - 1)  # DVE freed ps_c[g%2]
                for o in range(NT):
                    for k in range(NT):
                        mm = tensor.matmul(
                            out=ps_c[g % 2][:, o * 128:(o + 1) * 128],
                            lhsT=wr_s[:, k * H + o * 128:k * H + (o + 1) * 128],
                            rhs=et_s[:, etb(g) + k * 128:etb(g) + (k + 1) * 128],
                            start=(k == 0),
                            stop=(k == NT - 1),
                        )
                mm.then_inc(s_bb, 1)

            transposes(0)
            for g in range(GROUPS):
                if g + 1 < GROUPS:
                    transposes(g + 1)
                phase_b(g)

            # ---- phase C: sequential steps ----
            ct_v = ct_s[:, :].rearrange(
                "p (g o q b) -> p g o q b", g=GROUPS, o=NT, q=4, b=B_CORE)
            for i in range(1, STEPS + 1):
                g, pos = i // 4, i % 4
                cur, nxt = (i - 1) % 2, i % 2
                if i == 1:
                    tensor.wait_ge(s_ct, GROUPS)  # all CT ready
                # inject c_i via identity matmul (no hT dependency)
                tensor.matmul(
                    out=ps_h[nxt][:, 0:128],
                    lhsT=i16_s[:, :],
                    rhs=ct_v[:, g, :, pos, :],
                    start=True,
                    stop=False,
                )
                if i == 1:
                    tensor.wait_ge(s_eT, 1)  # hT0 seeded by DVE
                else:
                    tensor.wait_ge(s_tanh, i - 1)
                hb = cur * 128
                for k in range(NT):
                    for o in range(NT):
                        mm = tensor.matmul(
                            out=ps_h[nxt][:, o * 32:(o + 1) * 32],
                            lhsT=wl_s[:, k * H + o * 128:k * H + (o + 1) * 128],
                            rhs=ht_s[:, hb + k * 32:hb + (k + 1) * 32],
                            start=False,
                            stop=(k == NT - 1),
                        )
                mm.then_inc(s_mm, 1)

            # ---- final transpose back to natural [32, 512] ----
            tensor.wait_ge(s_tanh, STEPS)
            hb = (STEPS % 2) * 128
            for k in range(NT):
                mm = tensor.transpose(
                    out=ps_f[0:B_CORE, k * 128:(k + 1) * 128],
                    in_=ht_s[:, hb + k * 32:hb + (k + 1) * 32],
                    identity=i16_s[:, :],
                )
            mm.then_inc(s_mm, 1)

        # ------------- vector engine (DVE) -------------
        @block.vector
        def _(vector):
            vector.wait_ge(s_pre, 80)

            def cast(g):
                vector.wait_ge(s_gth, 16 * (g + 1))
                if g >= E16_RING:
                    vector.wait_ge(s_trA, g - E16_RING + 1)
                vector.tensor_copy(
                    out=e16_s[:, e16b(g):e16b(g) + H],
                    in_=e32_s[:, e32b(g):e32b(g) + H],
                ).then_inc(s_cast, 1)

            def copy_eT(g):
                vector.wait_ge(s_trA, g + 1)
                if g >= ET_RING:
                    vector.wait_ge(s_bb, g - ET_RING + 1)
                cp = vector.tensor_copy(
                    out=et_s[:, etb(g):etb(g) + H],
                    in_=ps_tr[g % 2][:, 0:H],
                )
                if g == 0:
                    # seed hT0: leaf 0 = flat columns 0..31 of group 0
                    et_v = et_s[:, :].rearrange(
                        "p (r k f) -> p r k f", r=ET_RING, k=NT, f=128)
                    ht_v = ht_s[:, 0:128].rearrange(
                        "p (k f) -> p k f", k=NT, f=B_CORE)
                    vector.tensor_copy(
                        out=ht_v,
                        in_=et_v[:, 0, :, 0:B_CORE],
                    ).then_inc(s_eT, 1)
                else:
                    cp.then_inc(s_eT, 1)

            def copy_ct(g):
                vector.wait_ge(s_bb, g + 1)
                for o in range(NT):
                    cc = vector.tensor_scalar_add(
                        out=ct_s[:, g * H + o * 128:g * H + (o + 1) * 128],
                        in0=ps_c[g % 2][:, o * 128:(o + 1) * 128],
                        scalar1=bt_s[:, o:o + 1],
                    )
                cc.then_inc(s_ct, 1)

            cast(0)
            cast(1)
            for g in range(GROUPS):
                copy_eT(g)
                if g + 2 < GROUPS:
                    cast(g + 2)
                copy_ct(g)

            vector.wait_ge(s_mm, STEPS + 1)
            vector.tensor_copy(out=y_s[:, :], in_=ps_f[0:B_CORE, 0:H]).then_inc(s_fdve, 1)

        # ------------- scalar engine (ACT): tanh -------------
        @block.scalar
        def _(scalar):
            for i in range(1, STEPS + 1):
                scalar.wait_ge(s_mm, i)
                scalar.activation(
                    out=ht_s[:, (i % 2) * 128:(i % 2) * 128 + 128],
                    in_=ps_h[i % 2][:, 0:128],
                    func=mybir.ActivationFunctionType.Tanh,
                ).then_inc(s_tanh, 1)

    return nc


def _numpy_fallback(tokens, emb, Wl, Wr, b):
    """Faithful numpy port of the reference stack machine (general tokens)."""
    PAD, OPEN, CLOSE = 2, 0, 1
    tokens = np.asarray(tokens, dtype=np.int64)
    T, B = tokens.shape
    Hd = emb.shape[1]
    S = T // 2 + 2
    op_mask = (tokens != PAD) & (tokens != OPEN)
    dest = np.cumsum(op_mask.astype(np.int64), axis=0) - 1
    op_input = np.full((T, B), PAD, dtype=np.int64)
    for col in range(B):
        src = tokens[op_mask[:, col], col]
        op_input[:len(src), col] = src
    close_mask = op_input == CLOSE
    token_mask = (op_input != PAD) & ~close_mask
    emb = np.asarray(emb, dtype=np.float32)
    Wl = np.asarray(Wl, dtype=np.float32)
    Wr = np.asarray(Wr, dtype=np.float32)
    b = np.asarray(b, dtype=np.float32)
    stack = np.zeros((B, S, Hd), dtype=np.float32)
    ptr = np.zeros(B, dtype=np.int64)
    bidx = np.arange(B)
    pos = np.arange(S)
    for t in range(T):
        e_t = emb[op_input[t]]
        is_shift = token_mask[t]
        is_reduce = close_mask[t]
        shift_oh = (pos[None, :] == ptr[:, None]) & is_shift[:, None]
        stack = np.where(shift_oh[..., None], e_t[:, None, :], stack)
        r_idx = np.clip(ptr - 1, 0, S - 1)
        l_idx = np.clip(ptr - 2, 0, S - 1)
        r_child = stack[bidx, r_idx]
        l_child = stack[bidx, l_idx]
        parent = np.tanh(l_child @ Wl + r_child @ Wr + b)
        reduce_oh = (pos[None, :] == l_idx[:, None]) & is_reduce[:, None]
        stack = np.where(reduce_oh[..., None], parent[:, None, :], stack)
        ptr = ptr + is_shift.astype(np.int64) - is_reduce.astype(np.int64)
    return stack[:, 0].astype(np.float32)


def _fast_path_ok(tokens):
    return (
        tokens.shape == (511, 256)
        and bool((tokens[0] >= 3).all())
        and bool((tokens[1::2] >= 3).all())
        and bool((tokens[2::2] == 1).all())
    )


def kernel(tokens, emb, Wl, Wr, b):
    tokens = np.asarray(tokens)
    emb = np.ascontiguousarray(np.asarray(emb, dtype=np.float32))
    Wl = np.asarray(Wl, dtype=np.float32)
    Wr = np.asarray(Wr, dtype=np.float32)
    b = np.asarray(b, dtype=np.float32)

    if not _fast_path_ok(tokens):
        return _numpy_fallback(tokens, emb, Wl, Wr, b)

    from concourse.bass_utils import run_bass_kernel_spmd

    if "nc" not in _cached:
        _cached["nc"] = _build_nc()
    nc = _cached["nc"]

    # leaf tokens: [256 leaves, 256 columns]
    L = np.concatenate([tokens[0:1], tokens[1::2]], axis=0).astype(np.int64)
    wl16 = np.ascontiguousarray(
        Wl.reshape(NT, 128, H).transpose(1, 0, 2).reshape(128, NT * H).astype(np.float16))
    wr16 = np.ascontiguousarray(
        Wr.reshape(NT, 128, H).transpose(1, 0, 2).reshape(128, NT * H).astype(np.float16))
    i16 = np.eye(128, dtype=np.float16)
    bt = np.ascontiguousarray(b.reshape(NT, 128).T.astype(np.float32))

    in_maps = []
    for c in range(N_CORES):
        Lc = L[:, c * B_CORE:(c + 1) * B_CORE]               # [256, 32]
        idx = np.ascontiguousarray(
            Lc.reshape(GROUPS, 4, B_CORE).transpose(1, 2, 0)
            .reshape(128, GROUPS).astype(np.int32))
        in_maps.append({
            "emb": emb, "wl16": wl16, "wr16": wr16,
            "i16": i16, "bt": bt, "idx": idx,
        })

    res = run_bass_kernel_spmd(nc, in_maps, core_ids=list(range(N_CORES)))
    out = np.concatenate([res.results[c]["y"] for c in range(N_CORES)], axis=0)
    return out.astype(np.float32)


# revision 16
# speedup vs baseline: 95.7189x; 95.7189x over previous
"""Trainium2 Bass kernel for the shift-reduce recursive NN (nn_Recursive_44040594653244).

Reference semantics: a per-column shift-reduce stack machine over tokens
[511, 256].  setup_inputs() always emits the structure
    row 0:        shift token   (>= 3)
    rows 1,3,...: shift tokens  (>= 3)
    rows 2,4,...: CLOSE (reduce)
with no PAD/OPEN, which makes the computation a left-leaning chain per
batch column b:
    h = emb[tok0[b]]
    for i in 1..255:  h = tanh(h @ Wl + emb[tok_i[b]] @ Wr + bias)
    out[b] = h                                    # [256, 512] fp32

Device strategy (pure batch data-parallel, 8 cores x 32 columns):
  - indirect-DMA gather of the 8192 leaf-embedding rows per core
  - DVE cast fp32 -> fp16, PE transposes to a hidden-major layout
  - phase B: C^T tiles = Wr^T @ E^T for all steps (fp16 operands, fp32 psum)
  - phase C: 255 sequential steps with state kept transposed as
    hT [128 part = hid-within-k-tile, free = k-tile(4) x batch(32)]:
      psum = I @ cT_i  +  sum_k Wl[k,o-tile]^T @ hT[k]   (17 matmuls)
      hT   = tanh(psum)                                  (1 ACT op)
  - final PE transpose back to natural [32, 512] fp32

fp16 storage with fp32 psum accumulation gives ~2e-3 relative error vs
the fp32 reference (verified offline in numpy simulation).
"""

import numpy as np

H = 512            # hidden size
NT = 4             # 128-wide tiles in H
B_CORE = 32        # batch columns per core
N_CORES = 8
GROUPS = 64        # gather groups: 4 leaves x 32 batch = 128 rows per group
STEPS = 255        # sequential reduce steps
E32_RING = 8       # gathered fp32 ring ([128, 512] tiles)
E16_RING = 4       # fp16-cast ring
ET_RING = 4        # transposed fp16 ring

_cached = {}


def _build_nc(mode='full', steps=None, amp=1):
    import concourse.bass as bass
    import concourse.mybir as mybir

    dt = mybir.dt
    steps = STEPS if steps is None else steps
    amp_ab = amp if mode in ('amp_ab', 'amp_ab_nogather', 'amp_all') else 1
    amp_c = amp if mode in ('amp_chain', 'amp_all') else 1
    n_groups_total = GROUPS * amp_ab
    n_ct_incs = n_groups_total
    nc = bass.Bass(target_bir_lowering=False)

    emb_d = nc.dram_tensor("emb", [32000, H], dt.float32, kind="ExternalInput")
    wl_d = nc.dram_tensor("wl16", [128, NT * H], dt.float16, kind="ExternalInput")
    wr_d = nc.dram_tensor("wr16", [128, NT * H], dt.float16, kind="ExternalInput")
    i16_d = nc.dram_tensor("i16", [128, 128], dt.float16, kind="ExternalInput")
    bt_d = nc.dram_tensor("bt", [128, NT], dt.float32, kind="ExternalInput")
    idx_d = nc.dram_tensor("idx", [128, GROUPS], dt.int32, kind="ExternalInput")
    y_d = nc.dram_tensor("y", [B_CORE, H], dt.float32, kind="ExternalOutput")

    from contextlib import ExitStack

    with ExitStack() as ctx:
        ent = ctx.enter_context
        wl_s = ent(nc.sbuf_tensor("wl16s", [128, NT * H], dt.float16))
        wr_s = ent(nc.sbuf_tensor("wr16s", [128, NT * H], dt.float16))
        i16_s = ent(nc.sbuf_tensor("i16s", [128, 128], dt.float16))
        bt_s = ent(nc.sbuf_tensor("bts", [128, NT], dt.float32))
        idx_s = ent(nc.sbuf_tensor("idxs", [128, GROUPS], dt.int32))
        e16_s = ent(nc.sbuf_tensor("e16", [128, GROUPS * H], dt.float16))
        et_s = ent(nc.sbuf_tensor("et16", [128, ET_RING * H], dt.float16))
        ct_s = ent(nc.sbuf_tensor("ct16", [128, GROUPS * H], dt.float16))
        ht_s = ent(nc.sbuf_tensor("ht16", [128, 2 * 128], dt.float16))
        y_s = ent(nc.sbuf_tensor("y32", [B_CORE, H], dt.float32))
        # each PSUM tensor is exactly one 2KB/partition bank
        # fp16 (transpose out dtype must match input); padded to a full bank
        ps_tr0 = ent(nc.psum_tensor("ps_tr0", [128, 1024], dt.float16))
        ps_tr1 = ent(nc.psum_tensor("ps_tr1", [128, 1024], dt.float16))
        ps_c0 = ent(nc.psum_tensor("ps_c0", [128, 512], dt.float32))
        ps_c1 = ent(nc.psum_tensor("ps_c1", [128, 512], dt.float32))
        ps_h0 = ent(nc.psum_tensor("ps_h0", [128, 512], dt.float32))
        ps_h1 = ent(nc.psum_tensor("ps_h1", [128, 512], dt.float32))
        ps_f = ent(nc.psum_tensor("ps_f", [128, 1024], dt.float16))
        s_pre = ent(nc.semaphore("s_pre"))     # input DMAs (sync), +16 each
        s_g = [ent(nc.semaphore(f"s_g{i}")) for i in range(E32_RING)]  # per-slot gather sems

        s_trA = ent(nc.semaphore("s_trA"))     # PE transposes, +1 per group
        s_eT = ent(nc.semaphore("s_eT"))       # DVE eT copy, +1 per group
        s_bb = ent(nc.semaphore("s_bb"))       # PE phase-B MMs, +1 per group
        s_ct = ent(nc.semaphore("s_ct"))       # DVE CT copy, +1 per group
        s_mm = ent(nc.semaphore("s_mm"))       # PE step MMs, +1 per step
        s_tanh = ent(nc.semaphore("s_tanh"))   # ACT tanh, +1 per step
        s_fdve = ent(nc.semaphore("s_fdve"))
        s_done = ent(nc.semaphore("s_done"))
        block = ent(nc.Block())
        ps_tr = [ps_tr0, ps_tr1]
        ps_c = [ps_c0, ps_c1]
        ps_h = [ps_h0, ps_h1]

        def e16b(g):
            return (g % GROUPS) * H

        def etb(g):
            return (g % ET_RING) * H

        # ------------- sync: input DMAs + final store -------------
        @block.sync
        def _(sync):
            sync.dma_start(idx_s[:, :], idx_d[:, :]).then_inc(s_pre, 16)
            sync.dma_start(i16_s[:, :], i16_d[:, :]).then_inc(s_pre, 16)
            sync.dma_start(wl_s[:, :], wl_d[:, :]).then_inc(s_pre, 16)
            sync.dma_start(wr_s[:, :], wr_d[:, :]).then_inc(s_pre, 16)
            sync.dma_start(bt_s[:, :], bt_d[:, :]).then_inc(s_pre, 16)
            sync.wait_ge(s_fdve, 1)
            sync.dma_start(y_d[:, :], y_s[:, :]).then_inc(s_done, 16)
            sync.wait_ge(s_done, 16)

        # ------------- gpsimd: fp16-cast indirect gathers -------------
        @block.gpsimd
        def _(gpsimd):
            gpsimd.wait_ge(s_pre, 16)  # idx loaded
            if mode in ('nogather', 'amp_ab_nogather'):
                gpsimd.memset(e16_s[:, :], 0.125)
                for g in range(n_groups_total):
                    gpsimd.sem_inc(s_g[g % E32_RING], 16)
                return
            for g in range(n_groups_total):
                if g >= E32_RING:
                    # issue-throttle: makes the per-slot sem counts attributable
                    # (gather g+8 is issued only after transposes(g) completed)
                    gpsimd.wait_ge(s_trA, g - E32_RING + 1)
                gpsimd.indirect_dma_start(
                    out=e16_s[:, e16b(g):e16b(g) + H],
                    out_offset=None,
                    in_=emb_d[:, :],
                    in_offset=bass.IndirectOffsetOnAxis(
                        ap=idx_s[:, (g % GROUPS):(g % GROUPS) + 1], axis=0),
                ).then_inc(s_g[g % E32_RING], 16)

        # ------------- tensor engine -------------
        @block.tensor
        def _(tensor):
            tensor.wait_ge(s_pre, 80)  # all inputs resident
            if mode == 'gatheronly':
                for i in range(E32_RING):
                    tensor.wait_ge(s_g[i], 16 * (GROUPS // E32_RING))
                for k in range(NT):
                    mm = tensor.transpose(
                        out=ps_f[0:B_CORE, k * 128:(k + 1) * 128],
                        in_=ht_s[:, k * 32:(k + 1) * 32],
                        identity=i16_s[:, :],
                    )
                mm.then_inc(s_mm, 1)
                return

            def transposes(g):
                # E16[g] [128 flat, 512 hid] -> psumT [128 hid-in-k, 4k x 128 flat]
                tensor.wait_ge(s_g[g % E32_RING], 16 * (g // E32_RING + 1))
                if g >= 2:
                    tensor.wait_ge(s_eT, g - 1)  # DVE freed ps_tr[g%2]
                for j in range(NT):
                    mm = tensor.transpose(
                        out=ps_tr[g % 2][:, j * 128:(j + 1) * 128],
                        in_=e16_s[:, e16b(g) + j * 128:e16b(g) + (j + 1) * 128],
                        identity=i16_s[:, :],
                    )
                mm.then_inc(s_trA, 1)


            # ---- piece queue: interleave group production into the chain ----
            def bmm_piece(g, o):
                # one o-block of phase B for group g
                if o == 0:
                    tensor.wait_ge(s_eT, g + 1)
                    if g >= 2:
                        tensor.wait_ge(s_ct, g - 1)  # DVE freed ps_c[g%2]
                for k in range(NT):
                    mm = tensor.matmul(
                        out=ps_c[g % 2][:, o * 128:(o + 1) * 128],
                        lhsT=wr_s[:, k * H + o * 128:k * H + (o + 1) * 128],
                        rhs=et_s[:, etb(g) + k * 128:etb(g) + (k + 1) * 128],
                        start=(k == 0),
                        stop=(k == NT - 1),
                    )
                if o == NT - 1:
                    mm.then_inc(s_bb, 1)

            def group_pieces(g):
                yield lambda: transposes(g)
                for o in range(NT):
                    yield lambda o=o: bmm_piece(g, o)

            PRO = 6  # groups fully produced before the chain starts
            if mode == 'full':
                pieces = []
                for g in range(PRO, n_groups_total):
                    pieces.extend(group_pieces(g))
                pc = 0  # emission cursor
                emitted_through = PRO - 1

                for g in range(PRO):
                    transposes(g)
                    for o in range(NT):
                        bmm_piece(g, o)
            else:
                transposes(0)
                for g in range(n_groups_total):
                    if g + 1 < n_groups_total:
                        transposes(g + 1)
                    for o in range(NT):
                        bmm_piece(g, o)

            # ---- phase C: sequential steps ----
            ct_v = ct_s[:, :].rearrange(
                "p (g o q b) -> p g o q b", g=GROUPS, o=NT, q=4, b=B_CORE)
            for i in range(1, amp_c * steps + 1):
                ie = (i - 1) % steps + 1
                g, pos = ie // 4, ie % 4
                cur, nxt = (i - 1) % 2, i % 2
                if mode == 'full':
                    if i == 1 or (ie % 4 == 0 and i == ie):
                        tensor.wait_ge(s_ct, min(g + 1, n_ct_incs))
                elif i == 1:
                    tensor.wait_ge(s_ct, n_ct_incs)  # all CT ready
                # inject c_i via identity matmul (no hT dependency)
                tensor.matmul(
                    out=ps_h[nxt][:, 0:128],
                    lhsT=i16_s[:, :],
                    rhs=ct_v[:, g, :, pos, :],
                    start=True,
                    stop=False,
                )
                if i == 1:
                    tensor.wait_ge(s_eT, 1)  # hT0 seeded by DVE
                else:
                    tensor.wait_ge(s_tanh, i - 1)
                hb = cur * 128
                for k in range(NT):
                    for o in range(NT):
                        mm = tensor.matmul(
                            out=ps_h[nxt][:, o * 32:(o + 1) * 32],
                            lhsT=wl_s[:, k * H + o * 128:k * H + (o + 1) * 128],
                            rhs=ht_s[:, hb + k * 32:hb + (k + 1) * 32],
                            start=False,
                            stop=(k == NT - 1),
                        )
                mm.then_inc(s_mm, 1)
                # emit production pieces during ACT_i: stay P groups ahead
                if mode == 'full' and i == ie:
                    want_through = min(n_groups_total - 1, ie // 4 + PRO)
                    while emitted_through < want_through and pc < len(pieces):
                        pieces[pc]()
                        pc += 1
                        if pc % (NT + 1) == 0:
                            emitted_through += 1
                    if ie == steps:
                        while pc < len(pieces):
                            pieces[pc]()
                            pc += 1

            # ---- final transpose back to natural [32, 512] ----
            tensor.wait_ge(s_tanh, amp_c * steps)
            hb = ((amp_c * steps) % 2) * 128
            for k in range(NT):
                mm = tensor.transpose(
                    out=ps_f[0:B_CORE, k * 128:(k + 1) * 128],
                    in_=ht_s[:, hb + k * 32:hb + (k + 1) * 32],
                    identity=i16_s[:, :],
                )
            mm.then_inc(s_mm, 1)

        # ------------- vector engine (DVE) -------------
        @block.vector
        def _(vector):
            vector.wait_ge(s_pre, 80)
            if mode == 'gatheronly':
                vector.wait_ge(s_mm, 1)
                vector.tensor_copy(out=y_s[:, :], in_=ps_f[0:B_CORE, 0:H]).then_inc(s_fdve, 1)
                return

            def copy_eT(g):
                vector.wait_ge(s_trA, g + 1)
                if g >= ET_RING:
                    vector.wait_ge(s_bb, g - ET_RING + 1)
                cp = vector.tensor_copy(
                    out=et_s[:, etb(g):etb(g) + H],
                    in_=ps_tr[g % 2][:, 0:H],
                )
                if g == 0:
                    # seed hT0: leaf 0 = flat columns 0..31 of group 0
                    et_v = et_s[:, :].rearrange(
                        "p (r k f) -> p r k f", r=ET_RING, k=NT, f=128)
                    ht_v = ht_s[:, 0:128].rearrange(
                        "p (k f) -> p k f", k=NT, f=B_CORE)
                    vector.tensor_copy(
                        out=ht_v,
                        in_=et_v[:, 0, :, 0:B_CORE],
                    ).then_inc(s_eT, 1)
                else:
                    cp.then_inc(s_eT, 1)

            def copy_ct(g):
                vector.wait_ge(s_bb, g + 1)
                gm = g % GROUPS
                for o in range(NT):
                    cc = vector.tensor_scalar_add(
                        out=ct_s[:, gm * H + o * 128:gm * H + (o + 1) * 128],
                        in0=ps_c[g % 2][:, o * 128:(o + 1) * 128],
                        scalar1=bt_s[:, o:o + 1],
                    )
                cc.then_inc(s_ct, 1)

            for g in range(n_groups_total):
                copy_eT(g)
                copy_ct(g)

            vector.wait_ge(s_mm, amp_c * steps + 1)
            vector.tensor_copy(out=y_s[:, :], in_=ps_f[0:B_CORE, 0:H]).then_inc(s_fdve, 1)

        # ------------- scalar engine (ACT): tanh -------------
        @block.scalar
        def _(scalar):
            if mode == 'gatheronly':
                return
            for i in range(1, amp_c * steps + 1):
                scalar.wait_ge(s_mm, i)
                scalar.activation(
                    out=ht_s[:, (i % 2) * 128:(i % 2) * 128 + 128],
                    in_=ps_h[i % 2][:, 0:128],
                    func=mybir.ActivationFunctionType.Tanh,
                ).then_inc(s_tanh, 1)

    return nc


def _numpy_fallback(tokens, emb, Wl, Wr, b):
    """Faithful numpy port of the reference stack machine (general tokens)."""
    PAD, OPEN, CLOSE = 2, 0, 1
    tokens = np.asarray(tokens, dtype=np.int64)
    T, B = tokens.shape
    Hd = emb.shape[1]
    S = T // 2 + 2
    op_mask = (tokens != PAD) & (tokens != OPEN)
    dest = np.cumsum(op_mask.astype(np.int64), axis=0) - 1
    op_input = np.full((T, B), PAD, dtype=np.int64)
    for col in range(B):
        src = tokens[op_mask[:, col], col]
        op_input[:len(src), col] = src
    close_mask = op_input == CLOSE
    token_mask = (op_input != PAD) & ~close_mask
    emb = np.asarray(emb, dtype=np.float32)
    Wl = np.asarray(Wl, dtype=np.float32)
    Wr = np.asarray(Wr, dtype=np.float32)
    b = np.asarray(b, dtype=np.float32)
    stack = np.zeros((B, S, Hd), dtype=np.float32)
    ptr = np.zeros(B, dtype=np.int64)
    bidx = np.arange(B)
    pos = np.arange(S)
    for t in range(T):
        e_t = emb[op_input[t]]
        is_shift = token_mask[t]
        is_reduce = close_mask[t]
        shift_oh = (pos[None, :] == ptr[:, None]) & is_shift[:, None]
        stack = np.where(shift_oh[..., None], e_t[:, None, :], stack)
        r_idx = np.clip(ptr - 1, 0, S - 1)
        l_idx = np.clip(ptr - 2, 0, S - 1)
        r_child = stack[bidx, r_idx]
        l_child = stack[bidx, l_idx]
        parent = np.tanh(l_child @ Wl + r_child @ Wr + b)
        reduce_oh = (pos[None, :] == l_idx[:, None]) & is_reduce[:, None]
        stack = np.where(reduce_oh[..., None], parent[:, None, :], stack)
        ptr = ptr + is_shift.astype(np.int64) - is_reduce.astype(np.int64)
    return stack[:, 0].astype(np.float32)


def _fast_path_ok(tokens):
    return (
        tokens.shape == (511, 256)
        and bool((tokens[0] >= 3).all())
        and bool((tokens[1::2] >= 3).all())
        and bool((tokens[2::2] == 1).all())
    )


def kernel(tokens, emb, Wl, Wr, b):
    tokens = np.asarray(tokens)
    emb = np.ascontiguousarray(np.asarray(emb, dtype=np.float32))
    Wl = np.asarray(Wl, dtype=np.float32)
    Wr = np.asarray(Wr, dtype=np.float32)
    b = np.asarray(b, dtype=np.float32)

    if not _fast_path_ok(tokens):
        return _numpy_fallback(tokens, emb, Wl, Wr, b)

    from concourse.bass_utils import run_bass_kernel_spmd

    if "nc" not in _cached:
        _cached["nc"] = _build_nc()
    nc = _cached["nc"]

    # leaf tokens: [256 leaves, 256 columns]
    L = np.concatenate([tokens[0:1], tokens[1::2]], axis=0).astype(np.int64)
    wl16 = np.ascontiguousarray(
        Wl.reshape(NT, 128, H).transpose(1, 0, 2).reshape(128, NT * H).astype(np.float16))
    wr16 = np.ascontiguousarray(
        Wr.reshape(NT, 128, H).transpose(1, 0, 2).reshape(128, NT * H).astype(np.float16))
    i16 = np.eye(128, dtype=np.float16)
    bt = np.ascontiguousarray(b.reshape(NT, 128).T.astype(np.float32))

    in_maps = []
    for c in range(N_CORES):
        Lc = L[:, c * B_CORE:(c + 1) * B_CORE]               # [256, 32]
        idx = np.ascontiguousarray(
            Lc.reshape(GROUPS, 4, B_CORE).transpose(1, 2, 0)
            .reshape(128, GROUPS).astype(np.int32))
        in_maps.append({
            "emb": emb, "wl16": wl16, "wr16": wr16,
            "i16": i16, "bt": bt, "idx": idx,
        })

    res = run_bass_kernel_spmd(nc, in_maps, core_ids=list(range(N_CORES)))
    out = np.concatenate([res.results[c]["y"] for c in range(N_CORES)], axis=0)
    return out.astype(np.float32)
